# revision 7
# baseline (speedup 1.0000x reference)
"""Trainium2 Bass kernel for nn_DepthLossV2 (N=8192 pairwise depth loss).

Math: with p = predictions[:,0], s = STEP*z_spacing*nth_slice, c = 0.2*s,
  steps[i,j] = |i-j|*s,  a[i,j] = p[i]-p[j]
  d = where(a>=0, a-0.2*steps, a); d = where(d>=0, max(d-0.8*steps,0), d)
  loss = sum(|tril(d)|)/N^2
On the tril region (j <= i, u = c*(i-j) >= 0) the summand separates:
  f = relu(q_i - q_j) + relu(r_i - r_j) - c*(i-j)*[p_j > p_i]
  with q_x = p_x - 5c*x, r_x = c*x - p_x.
The two relu terms are order-independent pairwise hinge sums — the Theta(N^2)
bulk — computed on device; the index-weighted inversion term is an exact
O(N log N) host correction (Fenwick tree), analogous to the wedge correction
a plain row-sharded kernel needs for its diagonal blocks.

Device layout (SPMD, 8 cores): transposed sharding — partitions hold a
128-wide tile of j (tile J = 8t + core for slot t = 0..7), the free dim
streams i. Slot t covers the compile-time-uniform stream m in [1024t, 8192);
per-core validity is enforced by DATA, not shapes: the streamed arrays are
q''[m + 128*core] padded with -60000 past the end, so out-of-range columns
contribute relu(negative) = 0 on every path. No wedge, no PSUM, no matmul.

Per column both hinge terms are needed; they are split between
  - DVE: one fused custom op  relu(Src0-C0) + relu(Src1-C1), ADD-accum
    (q-stream, r-stream, per-partition scalars q_j, r_j) at ~1.07 ns/col
  - Scalar engine: two Relu-activations with bias -q_j / -r_j and accum_out
    at ~0.92 ns/col each
with a static ~64/36 column split that balances the two engines.
Streams are fp16 (range-compressed by 1/4); accumulation is fp32.
"""

import os

import numpy as np

N = 8192
P = 128
NCORES = 8
SLOTS = 8
STEP = 1.0

PAD = -60000.0
QSCALE = 0.25           # q'' = q * QSCALE to fit fp16 range
ACT_FRAC = 0.32         # fraction of each slot's columns on the Scalar engine
DVE_CHUNK = 8192
ACT_CHUNK = 4096

_CACHE = {}
last_exec_ns = None
last_trace = None


def _register_qr_op():
    import concourse.dve_ops as dve_ops
    from concourse.dve_ops import DveOp, OPS
    from concourse.dve_spec import (
        Spec, Src0, Src1, C0, C1, AluOp, lower, relu, _has_src1,
    )
    from concourse.dve_uop import DveOpSpec

    name = "QR_RELU_SUM_ANT"
    if name in dve_ops._SUB_OPCODE_FOR_NAME:
        return next(op for op in OPS if op.name == name)

    body = relu(Src0 - C0) + relu(Src1 - C1)

    def ref(in0, in1, s0, s1, imm2):
        out = np.maximum(in0 - s0, 0.0) + np.maximum(in1 - s1, 0.0)
        return out, out.sum(axis=-1, keepdims=True)

    spec = Spec(body=body, accum=AluOp.ADD, reference=ref)
    row = dve_ops._CUSTOM_DVE_ROW_BASE + len(OPS)
    assert row < 0x20, "no free custom-DVE opcode rows"
    shas = {}
    for ver in ("v3", "v4"):
        d = DveOpSpec(name=name, opcode=row, uops=lower(spec, ver=ver),
                      rd1_en=_has_src1(spec))
        shas[ver] = d.sha(ver)
    op = DveOp(name, spec, subdim=False, uops_sha=shas)
    OPS.append(op)
    dve_ops._SUB_OPCODE_FOR_NAME[name] = row
    dve_ops.CUSTOM_DVE_SPECS[name] = spec
    return op


def _slot_split(t):
    """(start, dve_width, act_width) for slot t's stream [1024t, 8192)."""
    start = 1024 * t
    w = N - start
    act_w = int(round(ACT_FRAC * w / 512.0)) * 512
    act_w = max(512, min(act_w, w - 512))
    return start, w - act_w, act_w


def _build_program():
    import concourse.bacc as bacc
    import concourse.mybir as mybir
    import concourse.tile as tile

    qr_op = _register_qr_op()

    # count accum slots
    nacc = 0
    for t in range(SLOTS):
        _, dve_w, act_w = _slot_split(t)
        nacc += -(-dve_w // DVE_CHUNK)          # DVE chunks
        nacc += 2 * -(-act_w // ACT_CHUNK)      # ACT chunks (q pass + r pass)

    nc = bacc.Bacc(trn_type="TRN2", name="depthloss2")
    qr_d = nc.dram_tensor("qr", [P, 2 * N], mybir.dt.float16,
                          kind="ExternalInput")
    consts_d = nc.dram_tensor("consts", [P, 4 * SLOTS], mybir.dt.float32,
                              kind="ExternalInput")
    acc_d = nc.dram_tensor("acc", [P, nacc], mybir.dt.float32,
                           kind="ExternalOutput")

    with tile.TileContext(nc) as tc:
        with (
            tc.tile_pool(name="persist", bufs=1) as persist,
            tc.tile_pool(name="work", bufs=3) as work,
        ):
            consts_t = persist.tile([P, 4 * SLOTS], mybir.dt.float32)
            nc.sync.dma_start(consts_t[:], consts_d[:])

            # warm the ACT function table immediately (no DMA dependency)
            warm_in = persist.tile([P, 1], mybir.dt.float32)
            nc.vector.memset(warm_in[:], 0.0)
            warm_t = work.tile([P, 1], mybir.dt.float32, tag="warm")
            nc.scalar.activation(warm_t[:], warm_in[:],
                                 mybir.ActivationFunctionType.Relu,
                                 bias=0.0, scale=1.0)

            qr_t = persist.tile([P, 2 * N], mybir.dt.float16)
            # Chunked loads in slot-priority order (high m first): the q
            # stream rides the SP HWDGE queue, the r stream the Pool SWDGE
            # queue, so the two halves land in parallel and strictly in the
            # order compute consumes them. The Act queue stays clear of
            # input DMA — dma_starts there would stall the Relu dispatch.
            for (c0, c1) in ((7168, 8192), (4096, 7168), (0, 4096)):
                nc.sync.dma_start(qr_t[:, c0:c1], qr_d[:, c0:c1])
                nc.gpsimd.dma_start(qr_t[:, N + c0:N + c1],
                                    qr_d[:, N + c0:N + c1])

            acc_t = persist.tile([P, nacc], mybir.dt.float32)

            unit = 0
            for t in reversed(range(SLOTS)):
                start, dve_w, act_w = _slot_split(t)
                qj = consts_t[:, t:t + 1]
                rj = consts_t[:, SLOTS + t:SLOTS + t + 1]
                nqj = consts_t[:, 2 * SLOTS + t:2 * SLOTS + t + 1]
                nrj = consts_t[:, 3 * SLOTS + t:3 * SLOTS + t + 1]

                # DVE head
                off = start
                while off < start + dve_w:
                    cw = min(DVE_CHUNK, start + dve_w - off)
                    f_t = work.tile([P, DVE_CHUNK], mybir.dt.float16, tag="f")
                    nc.vector._custom_dve(
                        qr_op, out=f_t[:, :cw],
                        in0=qr_t[:, off:off + cw],
                        in1=qr_t[:, N + off:N + off + cw],
                        s0=qj, s1=rj,
                        accum_out=acc_t[:, unit:unit + 1])
                    unit += 1
                    off += cw
                # ACT tail: q pass + r pass
                a0 = start + dve_w
                for (base, nb) in ((0, nqj), (N, nrj)):
                    off = a0
                    while off < start + dve_w + act_w:
                        cw = min(ACT_CHUNK, start + dve_w + act_w - off)
                        g_t = work.tile([P, ACT_CHUNK], mybir.dt.float16,
                                        tag="g")
                        nc.scalar.activation(
                            g_t[:, :cw], qr_t[:, base + off:base + off + cw],
                            mybir.ActivationFunctionType.Relu,
                            bias=nb, scale=1.0,
                            accum_out=acc_t[:, unit:unit + 1])
                        unit += 1
                        off += cw

            assert unit == nacc
            nc.sync.dma_start(acc_d[:], acc_t[:])

    nc.compile()
    return nc, nacc


def _t3_host(p64, c):
    """c * sum_{j<i, p_j > p_i} (i - j), exact via Fenwick tree."""
    n = p64.shape[0]
    order = np.argsort(p64, kind="stable")
    rank = np.empty(n, dtype=np.int64)
    rank[order] = np.arange(n)
    cnt = np.zeros(n + 1)
    sj = np.zeros(n + 1)

    def upd(b, pos, v):
        pos += 1
        while pos <= n:
            b[pos] += v
            pos += pos & (-pos)

    def qry(b, pos):
        pos += 1
        s = 0.0
        while pos > 0:
            s += b[pos]
            pos -= pos & (-pos)
        return s

    # strict p_j > p_i: with ties, count only strictly-greater values.
    # rank_hi[i] = highest rank among values equal to p64[i]
    sorted_vals = p64[order]
    hi_of_rank = np.searchsorted(sorted_vals, sorted_vals, side="right") - 1
    tot_c = 0
    tot_j = 0.0
    t3 = 0.0
    for i in range(n):
        rk = int(hi_of_rank[rank[i]])
        c_le = qry(cnt, rk)
        s_le = qry(sj, rk)
        t3 += i * (tot_c - c_le) - (tot_j - s_le)
        upd(cnt, rank[i], 1.0)
        upd(sj, rank[i], float(i))
        tot_c += 1
        tot_j += float(i)
    return c * t3


def kernel(predictions, z_spacing, nth_slice):
    global last_exec_ns, last_trace
    p = np.asarray(predictions, dtype=np.float32).reshape(N)
    s = float(STEP) * float(np.asarray(z_spacing)) * float(np.asarray(nth_slice))

    if not (s >= 0.0) or not np.isfinite(s):
        # negative/NaN step never occurs with the reference setup; fall back
        # to exact host evaluation for robustness.
        p64 = p.astype(np.float64)
        i = np.arange(N, dtype=np.float64)
        st = np.abs(i[:, None] - i[None, :]) * s
        a = p64[:, None] - p64[None, :]
        d = np.where(a >= 0, a - 0.2 * st, a)
        d = np.where(d >= 0, np.maximum(d - 0.8 * st, 0.0), d)
        return np.float32(np.abs(np.tril(d)).sum() / (N * N))

    c = 0.2 * s
    if "prog" not in _CACHE:
        _CACHE["prog"] = _build_program()
    nc, nacc = _CACHE["prog"]

    p64 = p.astype(np.float64)
    idx = np.arange(N, dtype=np.float64)
    q = (p64 - 5.0 * c * idx) * QSCALE
    r = (c * idx - p64) * QSCALE

    in_maps = []
    for core in range(NCORES):
        sh = 128 * core
        qrow = np.full(N, PAD, np.float64)
        rrow = np.full(N, PAD, np.float64)
        qrow[:N - sh] = q[sh:]
        rrow[:N - sh] = r[sh:]
        qr = np.empty((P, 2 * N), np.float16)
        qr[:, :N] = qrow.astype(np.float16)[None, :]
        qr[:, N:] = rrow.astype(np.float16)[None, :]
        consts = np.empty((P, 4 * SLOTS), np.float32)
        for t in range(SLOTS):
            rows = slice(128 * (8 * t + core), 128 * (8 * t + core) + P)
            consts[:, t] = q[rows]
            consts[:, SLOTS + t] = r[rows]
            consts[:, 2 * SLOTS + t] = -q[rows]
            consts[:, 3 * SLOTS + t] = -r[rows]
        in_maps.append({"qr": qr, "consts": consts})

    from concourse.bass_utils import run_bass_kernel_spmd
    trace = bool(int(os.environ.get("DEPTH_TRACE", "0")))
    if trace:
        try:
            import antenv.axon_hooks  # noqa: F401
        except ImportError:
            trace = False
    res = run_bass_kernel_spmd(nc, in_maps, core_ids=list(range(NCORES)),
                               trace=trace)
    last_exec_ns = res.exec_time_ns
    last_trace = res.instructions_and_trace
    total = np.float64(0.0)
    for rr in res.results:
        total += rr["acc"].astype(np.float64).sum()

    loss = (total / QSCALE - _t3_host(p64, c)) / (N * N)
    return np.float32(loss)


# revision 8
# speedup vs baseline: 1.0363x; 1.0363x over previous
"""Trainium2 Bass kernel for nn_DepthLossV2 (N=8192 pairwise depth loss).

Math: with p = predictions[:,0], s = STEP*z_spacing*nth_slice, c = 0.2*s,
  steps[i,j] = |i-j|*s,  a[i,j] = p[i]-p[j]
  d = where(a>=0, a-0.2*steps, a); d = where(d>=0, max(d-0.8*steps,0), d)
  loss = sum(|tril(d)|)/N^2
On the tril region (j <= i, u = c*(i-j) >= 0) the summand separates:
  f = relu(q_i - q_j) + relu(r_i - r_j) - c*(i-j)*[p_j > p_i]
  with q_x = p_x - 5c*x, r_x = c*x - p_x.
The two relu terms are order-independent pairwise hinge sums — the Theta(N^2)
bulk — computed on device; the index-weighted inversion term is an exact
O(N log N) host correction (Fenwick tree), analogous to the wedge correction
a plain row-sharded kernel needs for its diagonal blocks.

Device layout (SPMD, 8 cores): transposed sharding — partitions hold a
128-wide tile of j (tile J = 8t + core for slot t = 0..7), the free dim
streams i. Slot t covers the compile-time-uniform stream m in [1024t, 8192);
per-core validity is enforced by DATA, not shapes: the streamed arrays are
q''[m + 128*core] padded with -60000 past the end, so out-of-range columns
contribute relu(negative) = 0 on every path. No wedge, no PSUM, no matmul.

Per column both hinge terms are needed; they are split between
  - DVE: one fused custom op  relu(Src0-C0) + relu(Src1-C1), ADD-accum
    (q-stream, r-stream, per-partition scalars q_j, r_j) at ~1.07 ns/col
  - Scalar engine: two Relu-activations with bias -q_j / -r_j and accum_out
    at ~0.92 ns/col each
with a static ~64/36 column split that balances the two engines.
Streams are fp16 (range-compressed by 1/4); accumulation is fp32.
"""

import os

import numpy as np

N = 8192
P = 128
NCORES = 8
SLOTS = 8
STEP = 1.0

PAD = -60000.0
QSCALE = 0.25           # q'' = q * QSCALE to fit fp16 range
ACT_FRAC = 0.32         # fraction of each slot's columns on the Scalar engine
DVE_CHUNK = 8192
ACT_CHUNK = 4096

_CACHE = {}
last_exec_ns = None
last_trace = None


def _register_qr_op():
    import concourse.dve_ops as dve_ops
    from concourse.dve_ops import DveOp, OPS
    from concourse.dve_spec import (
        Spec, Src0, Src1, C0, C1, AluOp, lower, relu, _has_src1,
    )
    from concourse.dve_uop import DveOpSpec

    name = "QR_RELU_SUM_ANT"
    if name in dve_ops._SUB_OPCODE_FOR_NAME:
        return next(op for op in OPS if op.name == name)

    body = relu(Src0 - C0) + relu(Src1 - C1)

    def ref(in0, in1, s0, s1, imm2):
        out = np.maximum(in0 - s0, 0.0) + np.maximum(in1 - s1, 0.0)
        return out, out.sum(axis=-1, keepdims=True)

    spec = Spec(body=body, accum=AluOp.ADD, reference=ref)
    row = dve_ops._CUSTOM_DVE_ROW_BASE + len(OPS)
    assert row < 0x20, "no free custom-DVE opcode rows"
    shas = {}
    for ver in ("v3", "v4"):
        d = DveOpSpec(name=name, opcode=row, uops=lower(spec, ver=ver),
                      rd1_en=_has_src1(spec))
        shas[ver] = d.sha(ver)
    op = DveOp(name, spec, subdim=False, uops_sha=shas)
    OPS.append(op)
    dve_ops._SUB_OPCODE_FOR_NAME[name] = row
    dve_ops.CUSTOM_DVE_SPECS[name] = spec
    return op


def _slot_split(t):
    """(start, dve_width, act_width) for slot t's stream [1024t, 8192)."""
    start = 1024 * t
    w = N - start
    act_w = int(round(ACT_FRAC * w / 512.0)) * 512
    act_w = max(512, min(act_w, w - 512))
    return start, w - act_w, act_w


def _build_program():
    import concourse.bacc as bacc
    import concourse.mybir as mybir
    import concourse.tile as tile

    qr_op = _register_qr_op()

    # count accum slots
    nacc = 0
    for t in range(SLOTS):
        _, dve_w, act_w = _slot_split(t)
        nacc += -(-dve_w // DVE_CHUNK)          # DVE chunks
        nacc += 2 * -(-act_w // ACT_CHUNK)      # ACT chunks (q pass + r pass)

    nc = bacc.Bacc(trn_type="TRN2", name="depthloss2")
    qr_d = nc.dram_tensor("qr", [P, 2 * N], mybir.dt.float16,
                          kind="ExternalInput")
    consts_d = nc.dram_tensor("consts", [P, 4 * SLOTS], mybir.dt.float32,
                              kind="ExternalInput")
    acc_d = nc.dram_tensor("acc", [P, nacc], mybir.dt.float32,
                           kind="ExternalOutput")

    with tile.TileContext(nc) as tc:
        with (
            tc.tile_pool(name="persist", bufs=1) as persist,
            tc.tile_pool(name="work", bufs=3) as work,
        ):
            consts_t = persist.tile([P, 4 * SLOTS], mybir.dt.float32)
            nc.sync.dma_start(consts_t[:], consts_d[:])

            # warm the ACT function table immediately (no DMA dependency)
            warm_in = persist.tile([P, 1], mybir.dt.float32)
            nc.vector.memset(warm_in[:], 0.0)
            warm_t = work.tile([P, 1], mybir.dt.float32, tag="warm")
            nc.scalar.activation(warm_t[:], warm_in[:],
                                 mybir.ActivationFunctionType.Relu,
                                 bias=0.0, scale=1.0)

            qr_t = persist.tile([P, 2 * N], mybir.dt.float16)
            # Chunked loads in slot-priority order (high m first): the q
            # stream rides the SP HWDGE queue, the r stream the Pool SWDGE
            # queue, so the two halves land in parallel and strictly in the
            # order compute consumes them. The Act queue stays clear of
            # input DMA — dma_starts there would stall the Relu dispatch.
            for (c0, c1) in ((7168, 8192), (6144, 7168), (4096, 6144),
                             (2048, 4096), (0, 2048)):
                nc.sync.dma_start(qr_t[:, c0:c1], qr_d[:, c0:c1])
                nc.gpsimd.dma_start(qr_t[:, N + c0:N + c1],
                                    qr_d[:, N + c0:N + c1])

            acc_t = persist.tile([P, nacc], mybir.dt.float32)

            unit = 0
            for t in reversed(range(SLOTS)):
                start, dve_w, act_w = _slot_split(t)
                qj = consts_t[:, t:t + 1]
                rj = consts_t[:, SLOTS + t:SLOTS + t + 1]
                nqj = consts_t[:, 2 * SLOTS + t:2 * SLOTS + t + 1]
                nrj = consts_t[:, 3 * SLOTS + t:3 * SLOTS + t + 1]

                # DVE head
                off = start
                while off < start + dve_w:
                    cw = min(DVE_CHUNK, start + dve_w - off)
                    f_t = work.tile([P, DVE_CHUNK], mybir.dt.float16, tag="f")
                    nc.vector._custom_dve(
                        qr_op, out=f_t[:, :cw],
                        in0=qr_t[:, off:off + cw],
                        in1=qr_t[:, N + off:N + off + cw],
                        s0=qj, s1=rj,
                        accum_out=acc_t[:, unit:unit + 1])
                    unit += 1
                    off += cw
                # ACT tail: q pass + r pass
                a0 = start + dve_w
                for (base, nb) in ((0, nqj), (N, nrj)):
                    off = a0
                    while off < start + dve_w + act_w:
                        cw = min(ACT_CHUNK, start + dve_w + act_w - off)
                        g_t = work.tile([P, ACT_CHUNK], mybir.dt.float16,
                                        tag="g")
                        nc.scalar.activation(
                            g_t[:, :cw], qr_t[:, base + off:base + off + cw],
                            mybir.ActivationFunctionType.Relu,
                            bias=nb, scale=1.0,
                            accum_out=acc_t[:, unit:unit + 1])
                        unit += 1
                        off += cw

            assert unit == nacc
            nc.sync.dma_start(acc_d[:], acc_t[:])

    nc.compile()
    return nc, nacc


def _t3_host(p64, c):
    """c * sum_{j<i, p_j > p_i} (i - j), exact via Fenwick tree."""
    n = p64.shape[0]
    order = np.argsort(p64, kind="stable")
    rank = np.empty(n, dtype=np.int64)
    rank[order] = np.arange(n)
    cnt = np.zeros(n + 1)
    sj = np.zeros(n + 1)

    def upd(b, pos, v):
        pos += 1
        while pos <= n:
            b[pos] += v
            pos += pos & (-pos)

    def qry(b, pos):
        pos += 1
        s = 0.0
        while pos > 0:
            s += b[pos]
            pos -= pos & (-pos)
        return s

    # strict p_j > p_i: with ties, count only strictly-greater values.
    # rank_hi[i] = highest rank among values equal to p64[i]
    sorted_vals = p64[order]
    hi_of_rank = np.searchsorted(sorted_vals, sorted_vals, side="right") - 1
    tot_c = 0
    tot_j = 0.0
    t3 = 0.0
    for i in range(n):
        rk = int(hi_of_rank[rank[i]])
        c_le = qry(cnt, rk)
        s_le = qry(sj, rk)
        t3 += i * (tot_c - c_le) - (tot_j - s_le)
        upd(cnt, rank[i], 1.0)
        upd(sj, rank[i], float(i))
        tot_c += 1
        tot_j += float(i)
    return c * t3


def kernel(predictions, z_spacing, nth_slice):
    global last_exec_ns, last_trace
    p = np.asarray(predictions, dtype=np.float32).reshape(N)
    s = float(STEP) * float(np.asarray(z_spacing)) * float(np.asarray(nth_slice))

    if not (s >= 0.0) or not np.isfinite(s):
        # negative/NaN step never occurs with the reference setup; fall back
        # to exact host evaluation for robustness.
        p64 = p.astype(np.float64)
        i = np.arange(N, dtype=np.float64)
        st = np.abs(i[:, None] - i[None, :]) * s
        a = p64[:, None] - p64[None, :]
        d = np.where(a >= 0, a - 0.2 * st, a)
        d = np.where(d >= 0, np.maximum(d - 0.8 * st, 0.0), d)
        return np.float32(np.abs(np.tril(d)).sum() / (N * N))

    c = 0.2 * s
    if "prog" not in _CACHE:
        _CACHE["prog"] = _build_program()
    nc, nacc = _CACHE["prog"]

    p64 = p.astype(np.float64)
    idx = np.arange(N, dtype=np.float64)
    q = (p64 - 5.0 * c * idx) * QSCALE
    r = (c * idx - p64) * QSCALE

    in_maps = []
    for core in range(NCORES):
        sh = 128 * core
        qrow = np.full(N, PAD, np.float64)
        rrow = np.full(N, PAD, np.float64)
        qrow[:N - sh] = q[sh:]
        rrow[:N - sh] = r[sh:]
        qr = np.empty((P, 2 * N), np.float16)
        qr[:, :N] = qrow.astype(np.float16)[None, :]
        qr[:, N:] = rrow.astype(np.float16)[None, :]
        consts = np.empty((P, 4 * SLOTS), np.float32)
        for t in range(SLOTS):
            rows = slice(128 * (8 * t + core), 128 * (8 * t + core) + P)
            consts[:, t] = q[rows]
            consts[:, SLOTS + t] = r[rows]
            consts[:, 2 * SLOTS + t] = -q[rows]
            consts[:, 3 * SLOTS + t] = -r[rows]
        in_maps.append({"qr": qr, "consts": consts})

    from concourse.bass_utils import run_bass_kernel_spmd
    trace = bool(int(os.environ.get("DEPTH_TRACE", "0")))
    if trace:
        try:
            import antenv.axon_hooks  # noqa: F401
        except ImportError:
            trace = False
    res = run_bass_kernel_spmd(nc, in_maps, core_ids=list(range(NCORES)),
                               trace=trace)
    last_exec_ns = res.exec_time_ns
    last_trace = res.instructions_and_trace
    total = np.float64(0.0)
    for rr in res.results:
        total += rr["acc"].astype(np.float64).sum()

    loss = (total / QSCALE - _t3_host(p64, c)) / (N * N)
    return np.float32(loss)
